# revision 38
# baseline (speedup 1.0000x reference)
"""DoubleAttention forward on 8 Trainium2 NeuronCores.

Reference (per sample, x: [512, 4096] after flattening h*w):
    A = wA @ x + bA            [128, n]
    B = wB @ x + bB            [128, n]
    V = wV @ x + bV            [128, n]
    M = softmax(B, axis=ch)    [128, n]
    W = softmax(V, axis=ch)    [128, n]
    gd = A @ M.T               [128, 128]
    Z = gd @ W                 [128, n]
    out = wR @ Z + bR          [512, n]

Sharding: data-parallel over batch, 16 samples -> 8 cores x 2 each.

Schedule (all matmuls fp16; PE stream floor ~62us at 2.4GHz, which is
the binding constraint -- DMA is ~50us, elementwise ~45us/engine):
  - work is organized in n-chunks of 512 positions (4 pos-tiles):
    per chunk one [128,1024] PSUM quad holds A|B for 4 tiles, evacuated
    by ONE wide exp-ACT (scalar), ONE reduce (DVE), one A-scale (DVE):
    wide ops amortize the 150-450ns fixed per-op engine overheads.
  - V^T computed directly in [ch, pos] layout (weight chunks stationary)
    so phase 3 needs NO PE transposes: out = G @ Wn.  Softmax-V partition
    sums via ones128 matmul (sums replicated across partitions),
    reciprocal_approx_fast (5x faster than reciprocal), normalize on the
    otherwise-idle gpsimd (the only PSUM-free elementwise op here --
    GPSIMD cannot touch PSUM on TRN2).
  - softmax-B folded into the A-quad evacuation (per-partition scale).
  - gd accumulates on PE across chunks (1-chunk emission lag).
  - out = (wR gd) @ Wn: G^T = gd^T @ wR^T, then 32 512-col matmuls per
    sample rotating over 4 PSUM slots; evacuation split scalar/DVE.
  - DMA: host pre-packs x/out so every transfer is [128 part x 4KB
    contiguous].  Loads up-front on the sync HWDGE queue (first chunk
    on the scalar queue, in parallel); stores per-chunk on sync.
  - startup: dummy matmuls + a dummy exp during the ~6us framework
    prologue hold the PE p-state ramp (0.65->1.2->2.4GHz after 3us
    continuous busy) and preload the scalar EXP table off the
    critical path.
  - samples interleaved: s1 phase-1 chunks pad s0's gd tail and s0's
    out-phase evacuation drain.
"""

import sys

if "/opt/trn_rl_repo" not in sys.path:
    sys.path.insert(0, "/opt/trn_rl_repo")

import numpy as np

import concourse.bacc as bacc
import concourse.tile as tile
from concourse import mybir
from concourse.bass_utils import run_bass_kernel_spmd

N_CORES = 8
B_GLOBAL = 16
B_LOC = B_GLOBAL // N_CORES
C_IN, C_M, C_N = 512, 128, 128
H = W = 64
N = H * W                      # 4096 spatial positions
KC = C_IN // 128               # 4 contraction chunks
NG = 8                         # 8 n-chunks of 512 positions
SHIFT = float(-12.0 * np.log(2.0))   # exp downshift so fp16 never overflows
F32 = mybir.dt.float32
F16 = mybir.dt.float16
EXP = mybir.ActivationFunctionType.Exp
IDENT = mybir.ActivationFunctionType.Identity
AXX = mybir.AxisListType.X


def _build(has_bias_abv: bool, has_bias_r: bool):
    nc = bacc.Bacc("TRN2", target_bir_lowering=False, debug=False)

    # x / out packed host-side as [s, g, p, k, n']: 4KB-contiguous rows.
    x_d = nc.dram_tensor("x", (B_LOC, NG, 128, KC, 512), F16, kind="ExternalInput")
    # weights packed so partition rows are contiguous too
    wab_d = nc.dram_tensor("wab", (128, KC, 256), F16, kind="ExternalInput")
    wvt_d = nc.dram_tensor("wvt", (128, KC, 128), F16, kind="ExternalInput")
    wrt_d = nc.dram_tensor("wrt", (128, C_IN), F16, kind="ExternalInput")
    if has_bias_abv:
        bab_d = nc.dram_tensor("bab", (1, 256), F16, kind="ExternalInput")
        bvt_d = nc.dram_tensor("bvt", (128, 1), F32, kind="ExternalInput")
    if has_bias_r:
        brt_d = nc.dram_tensor("brt", (128, KC), F32, kind="ExternalInput")
    out_d = nc.dram_tensor("out", (B_LOC, NG, 128, KC, 512), F16, kind="ExternalOutput")

    with tile.TileContext(nc) as tc:
        with (
            tc.tile_pool(name="const", bufs=1) as constp,
            tc.tile_pool(name="xq", bufs=B_LOC * NG) as xqp,
            tc.tile_pool(name="mwb", bufs=5) as mwp,
            tc.tile_pool(name="at", bufs=5) as atp,
            tc.tile_pool(name="st", bufs=6) as stp,
            tc.tile_pool(name="ev", bufs=2) as evp,
            tc.tile_pool(name="wn", bufs=2) as wnp,
            tc.tile_pool(name="rw", bufs=4) as rwp,
            tc.tile_pool(name="gds", bufs=2) as gdsp,
            tc.tile_pool(name="gts", bufs=2) as gtsp,
            tc.tile_pool(name="osb", bufs=4) as osbp,
            tc.tile_pool(name="pP", bufs=2, space="PSUM") as pP,
            tc.tile_pool(name="pV", bufs=1, space="PSUM") as pV,
            tc.tile_pool(name="pG", bufs=1, space="PSUM") as pG,
            tc.tile_pool(name="pO", bufs=2, space="PSUM") as pO,
        ):
            # ---- warmup consts (no DMA deps) ----
            ones128 = constp.tile([128, 128], F16)
            nc.gpsimd.memset(ones128[:], 1.0)
            shiftv = constp.tile([128, 1], F32)
            nc.gpsimd.memset(shiftv[:], SHIFT)
            warm16 = constp.tile([128, 512], F16)
            nc.gpsimd.memset(warm16[:], 0.0)

            # ---- loads: first chunk on scalar queue, rest on sync ----
            xqs = [[None] * NG for _ in range(B_LOC)]

            def load_chunk(s, g, eng):
                t = xqp.tile([128, KC, 512], F16, tag="xq", name=f"xq{s}_{g}")
                eng.dma_start(t[:], x_d[s, g])
                xqs[s][g] = t

            # weights stream on sync's ring; the first two x chunks go on
            # scalar's ring in parallel (both rings start ~8.8-10.2us
            # after the fixed framework prologue).  Measured repeatedly:
            # putting x chunk 0 ahead of the weights on sync is ~3us SLOWER.
            load_chunk(0, 0, nc.scalar)
            wab = constp.tile([128, KC, 256], F16)
            nc.sync.dma_start(wab[:], wab_d[:])
            wvt = constp.tile([128, KC, 128], F16)
            nc.sync.dma_start(wvt[:], wvt_d[:])
            load_chunk(0, 1, nc.scalar)
            wrt = constp.tile([128, C_IN], F16)
            nc.sync.dma_start(wrt[:], wrt_d[:])
            for g in range(2, NG):
                load_chunk(0, g, nc.sync)
            for g in range(NG):
                load_chunk(1, g, nc.sync)

            if has_bias_abv:
                bab = constp.tile([1, 256], F16)
                nc.sync.dma_start(bab[:], bab_d[:])
                ones1 = constp.tile([1, 128], F16)
                nc.gpsimd.memset(ones1[:], 1.0)
                bvt = constp.tile([128, 1], F32)
                nc.sync.dma_start(bvt[:], bvt_d[:])
                shiftbv = constp.tile([128, 1], F32)
                nc.vector.tensor_scalar_add(shiftbv[:], bvt[:], SHIFT)
                vbias = shiftbv
            else:
                vbias = shiftv
            if has_bias_r:
                brt = constp.tile([128, KC], F32)
                nc.sync.dma_start(brt[:], brt_d[:])

            # ---- PE p-state warmup + EXP table preload during the DMA
            # wait (PE is free to run ~7.5us in; x chunk 0 lands ~11us)
            warmp = pG.tile([128, 512], F32, tag="pg", name="warmp")

            def dummy_mms(n, target=None):
                t = warmp if target is None else target
                for _ in range(n):
                    nc.tensor.matmul(t[:], ones128[:], warm16[:],
                                     start=True, stop=True)

            dummy_mms(13)
            wexp = constp.tile([128, 1], F16)
            nc.scalar.activation(wexp[:], shiftv[:], EXP, bias=0.0)

            # per-sample state
            mwB = [[None] * NG, [None] * NG]   # exp(B) fp16 [128, 4, 128]
            ats = [[None] * NG, [None] * NG]   # A/sumB fp16 [128, 4, 128]
            evt = [None, None]                 # exp(V^T) fp16 [128, NG, 512]
            wnt = [None, None]                 # normalized W^T fp16
            gdt = [None, None]                 # gd psum
            gts = [None, None]                 # G^T fp16 [128, 512]

            def emit_quad(s, g):
                """A|B projections for the 4 tiles of chunk g + softmax-B."""
                xq = xqs[s][g]
                p1 = pP.tile([128, 1024], F32, tag="pp", name=f"pp{s}_{g}")
                p1v = p1.rearrange("p (t a c) -> p t a c", t=4, c=128)
                for t in range(4):
                    dst = p1[:, t * 256:t * 256 + 256]
                    for k in range(KC):
                        nc.tensor.matmul(
                            dst, xq[:, k, t * 128:t * 128 + 128], wab[:, k, :],
                            start=(k == 0),
                            stop=(k == KC - 1 and not has_bias_abv),
                        )
                    if has_bias_abv:
                        nc.tensor.matmul(dst, ones1[:], bab[:],
                                         start=False, stop=True)
                # one wide exp + one wide reduce + one wide A-scale
                mw = mwp.tile([128, 4, 128], F16, tag="mwb", name=f"mw{s}_{g}")
                nc.scalar.activation(mw[:], p1v[:, :, 1, :], EXP, bias=shiftv[:])
                sums = stp.tile([128, 4, 1], F32, tag="sums")
                nc.vector.tensor_reduce(sums[:], mw[:], axis=AXX,
                                        op=mybir.AluOpType.add)
                rec = stp.tile([128, 4, 1], F32, tag="rec")
                nc.vector.reciprocal(rec[:], sums[:])
                at = atp.tile([128, 4, 128], F16, tag="at", name=f"at{s}_{g}")
                nc.vector.tensor_mul(at[:], p1v[:, :, 0, :],
                                     rec[:].broadcast_to([128, 4, 128]))
                mwB[s][g] = mw
                ats[s][g] = at

            def emit_vt(s, g):
                """V^T [ch, pos] projection for n-chunk g + exp."""
                pv = pV.tile([128, 512], F32, tag="pv", name=f"pv{s}_{g}")
                for k in range(KC):
                    nc.tensor.matmul(pv[:], wvt[:, k, :], xqs[s][g][:, k, :],
                                     start=(k == 0), stop=(k == KC - 1))
                nc.scalar.activation(evt[s][:, g, :], pv[:], EXP, bias=vbias[:])

            def emit_vsum(s, g):
                """Partition sums of exp(V^T) via ones-matmul; recip; Wn."""
                ps = pO.tile([128, 512], F32, tag="po", name=f"ps{s}_{g}")
                nc.tensor.matmul(ps[:], ones128[:], evt[s][:, g, :],
                                 start=True, stop=True)
                rw = rwp.tile([128, 512], F32, tag="rw", name=f"rw{s}_{g}")
                nc.vector.reciprocal_approx_fast(rw[:], ps[:])
                # fp16 SBUF-only multiply on the otherwise-idle gpsimd
                nc.gpsimd.tensor_mul(wnt[s][:, g, :], evt[s][:, g, :], rw[:])

            def emit_gd(s, g, first, last):
                for t in range(4):
                    nc.tensor.matmul(
                        gdt[s][:], ats[s][g][:, t, :], mwB[s][g][:, t, :],
                        start=(first and t == 0), stop=(last and t == 3),
                        skip_group_check=True,
                    )

            def emit_chunk(s, g):
                """All phase-1 work tied to x chunk g of sample s."""
                if g == 0:
                    evt[s] = evp.tile([128, NG, 512], F16, tag="ev",
                                      name=f"ev{s}")
                    wnt[s] = wnp.tile([128, NG, 512], F16, tag="wn",
                                      name=f"wn{s}")
                emit_quad(s, g)
                emit_vt(s, g)
                if g >= 1:
                    emit_vsum(s, g - 1)
                if g >= 2:
                    if g == 2:
                        gdt[s] = pG.tile([128, 128], F32, tag="pg",
                                         name=f"gd{s}")
                    emit_gd(s, g - 2, first=(g == 2), last=False)

            warm2 = [None]

            def emit_tail(s):
                """Finish gd, compute G^T = gd^T @ wR^T."""
                emit_vsum(s, NG - 1)
                emit_gd(s, NG - 2, first=False, last=False)
                emit_gd(s, NG - 1, first=False, last=True)
                gdts = gdsp.tile([128, 128], F16, tag="gds", name=f"gds{s}")
                nc.vector.tensor_copy(gdts[:], gdt[s][:])
                gtp = pO.tile([128, 512], F32, tag="po", name=f"gtp{s}")
                nc.tensor.matmul(gtp[:], gdts[:], wrt[:], start=True, stop=True)
                gt = gtsp.tile([128, 512], F16, tag="gts", name=f"gt{s}")
                nc.scalar.copy(gt[:], gtp[:])
                gts[s] = gt

            def bias_evac(dst, ock, b):
                nc.scalar.activation(dst, ock, IDENT, bias=brt[:, b:b + 1])

            def emit_out(s, gs):
                """out chunks (interleaved with s1 phase-1 while s=0):
                4 c-block matmuls, evac split b0/b1->scalar b2/b3->DVE,
                rotating over 4 narrow psum slots (pG owned by s1's gd)."""
                for g in gs:
                    osb = osbp.tile([128, KC, 512], F16, tag="osb",
                                    name=f"osb{s}_{g}")
                    for b in range(KC):
                        pool, ptag = ((pO, "po"), (pV, "pv"),
                                      (pO, "po"), (pV, "pv"))[b]
                        ock = pool.tile([128, 512], F32, tag=ptag,
                                        name=f"ock{s}_{g}_{b}")
                        nc.tensor.matmul(ock[:], gts[s][:, b * 128:b * 128 + 128],
                                         wnt[s][:, g, :], start=True, stop=True)
                        dst = osb[:, b, :]
                        if has_bias_r:
                            bias_evac(dst, ock[:], b)
                        elif b < 3:
                            # 3:1 scalar-heavy split: in the interleaved
                            # region DVE already carries the phase-1 chain
                            # (reduce+recip+at+recw) and would pace it
                            nc.scalar.copy(dst, ock[:])
                        else:
                            nc.vector.tensor_copy(dst, ock[:])
                    nc.sync.dma_start(out_d[s, g], osb[:])

            def emit_out_final(s):
                """Final out phase (nothing left to interleave): narrow
                evacs split b0/b1->scalar b2/b3->DVE; gd is done so pG
                serves as the 4th rotating psum slot.  The last chunk
                stores b-granular to cut the drain tail."""
                for g in range(NG):
                    osb = osbp.tile([128, KC, 512], F16, tag="osb",
                                    name=f"osb{s}_{g}")
                    for b in range(KC):
                        pool, ptag = ((pO, "po"), (pV, "pv"),
                                      (pO, "po"), (pG, "pg"))[b]
                        ock = pool.tile([128, 512], F32, tag=ptag,
                                        name=f"ock{s}_{g}_{b}")
                        nc.tensor.matmul(ock[:], gts[s][:, b * 128:b * 128 + 128],
                                         wnt[s][:, g, :], start=True, stop=True)
                        dst = osb[:, b, :]
                        if has_bias_r:
                            bias_evac(dst, ock[:], b)
                        elif b < 2:
                            nc.scalar.copy(dst, ock[:])
                        else:
                            nc.vector.tensor_copy(dst, ock[:])
                    if g < NG - 1:
                        nc.sync.dma_start(out_d[s, g], osb[:])
                    else:
                        nc.sync.dma_start(out_d[s, g, :, 0:2], osb[:, 0:2, :])
                        nc.sync.dma_start(out_d[s, g, :, 2:3], osb[:, 2:3, :])
                        nc.sync.dma_start(out_d[s, g, :, 3:4], osb[:, 3:4, :])

            # ---- schedule: s1 phase-1 interleaves s0's out phase ----
            for g in range(NG):
                emit_chunk(0, g)
            emit_chunk(1, 0)          # pads s0's gd-tail wait
            emit_tail(0)
            emit_chunk(1, 1)          # pads s0's G -> gts chain
            for g in range(2, NG):
                if g >= 3:
                    emit_out(0, [g - 3])
                emit_chunk(1, g)
            emit_out(0, [NG - 3])     # these two pad s1's at(7) softmax
            emit_out(0, [NG - 2])     # chain (~2.3us) before the gd tail
            emit_tail(1)
            emit_out(0, [NG - 1])     # pads s1's G -> gts chain
            emit_out_final(1)

    nc.compile()
    return nc


_CACHE = {}


def _get_nc(has_bias_abv: bool, has_bias_r: bool):
    key = (has_bias_abv, has_bias_r)
    if key not in _CACHE:
        _CACHE[key] = _build(*key)
    return _CACHE[key]


def _run(inputs, trace=False, **spmd_kwargs):
    x = np.asarray(inputs["x"])
    b, c, h, w = x.shape
    assert (b, c, h, w) == (B_GLOBAL, C_IN, H, W), x.shape
    wA = np.asarray(inputs["wA"], np.float32)
    wB = np.asarray(inputs["wB"], np.float32)
    wV = np.asarray(inputs["wV"], np.float32)
    wR = np.asarray(inputs["wR"], np.float32)
    bA = np.asarray(inputs["bA"], np.float32)
    bB = np.asarray(inputs["bB"], np.float32)
    bV = np.asarray(inputs["bV"], np.float32)
    bR = np.asarray(inputs["bR"], np.float32)

    has_bias_abv = bool(np.any(bA) or np.any(bB) or np.any(bV))
    has_bias_r = bool(np.any(bR))
    nc = _get_nc(has_bias_abv, has_bias_r)

    # [128, KC, 256]: row p, chunk k holds [wA.T | wB.T][k*128+p, :]
    wab = np.concatenate([wA.T, wB.T], axis=1).reshape(KC, 128, 256)
    wvt = wV.T.reshape(KC, 128, 128)
    base = {
        "wab": np.ascontiguousarray(wab.transpose(1, 0, 2), dtype=np.float16),
        "wvt": np.ascontiguousarray(wvt.transpose(1, 0, 2), dtype=np.float16),
        "wrt": np.ascontiguousarray(wR.T, dtype=np.float16),
    }
    if has_bias_abv:
        base["bab"] = np.concatenate([bA, bB])[None, :].astype(np.float16)
        base["bvt"] = np.ascontiguousarray(bV[:, None], np.float32)
    if has_bias_r:
        base["brt"] = np.ascontiguousarray(bR.reshape(KC, 128).T, np.float32)

    # pack x: [B, 512, 4096] -> [B, g, p, k, n']  (4KB contiguous rows)
    xh = (
        np.asarray(x, np.float16)
        .reshape(B_GLOBAL, KC, 128, NG, 512)
        .transpose(0, 3, 2, 1, 4)
    )
    in_maps = [
        dict(base, x=np.ascontiguousarray(
            xh[ci * B_LOC:(ci + 1) * B_LOC]))
        for ci in range(N_CORES)
    ]
    res = run_bass_kernel_spmd(
        nc, in_maps, core_ids=list(range(N_CORES)), trace=trace, **spmd_kwargs
    )
    # unpack out: [B_LOC, g, p, k, n'] -> [B_LOC, 512, 4096]
    outs = []
    for ci in range(N_CORES):
        o = res.results[ci]["out"].astype(np.float32)
        outs.append(o.transpose(0, 3, 2, 1, 4).reshape(B_LOC, C_IN, N))
    out = np.concatenate(outs, axis=0)
    return out.reshape(B_GLOBAL, C_IN, H, W), res


def kernel(**inputs):
    out, _ = _run(inputs)
    return out


# revision 39
# speedup vs baseline: 1.0192x; 1.0192x over previous
"""DoubleAttention forward on 8 Trainium2 NeuronCores.

Reference (per sample, x: [512, 4096] after flattening h*w):
    A = wA @ x + bA            [128, n]
    B = wB @ x + bB            [128, n]
    V = wV @ x + bV            [128, n]
    M = softmax(B, axis=ch)    [128, n]
    W = softmax(V, axis=ch)    [128, n]
    gd = A @ M.T               [128, 128]
    Z = gd @ W                 [128, n]
    out = wR @ Z + bR          [512, n]

Sharding: data-parallel over batch, 16 samples -> 8 cores x 2 each.

Schedule (all matmuls fp16; PE stream floor ~62us at 2.4GHz, which is
the binding constraint -- DMA is ~50us, elementwise ~45us/engine):
  - work is organized in n-chunks of 512 positions (4 pos-tiles):
    per chunk one [128,1024] PSUM quad holds A|B for 4 tiles, evacuated
    by ONE wide exp-ACT (scalar), ONE reduce (DVE), one A-scale (DVE):
    wide ops amortize the 150-450ns fixed per-op engine overheads.
  - V^T computed directly in [ch, pos] layout (weight chunks stationary)
    so phase 3 needs NO PE transposes: out = G @ Wn.  Softmax-V partition
    sums via ones128 matmul (sums replicated across partitions),
    reciprocal_approx_fast (5x faster than reciprocal), normalize on the
    otherwise-idle gpsimd (the only PSUM-free elementwise op here --
    GPSIMD cannot touch PSUM on TRN2).
  - softmax-B folded into the A-quad evacuation (per-partition scale).
  - gd accumulates on PE across chunks (1-chunk emission lag).
  - out = (wR gd) @ Wn: G^T = gd^T @ wR^T, then 32 512-col matmuls per
    sample rotating over 4 PSUM slots; evacuation split scalar/DVE.
  - DMA: host pre-packs x/out so every transfer is [128 part x 4KB
    contiguous].  Loads up-front on the sync HWDGE queue (first chunk
    on the scalar queue, in parallel); stores per-chunk on sync.
  - startup: dummy matmuls + a dummy exp during the ~6us framework
    prologue hold the PE p-state ramp (0.65->1.2->2.4GHz after 3us
    continuous busy) and preload the scalar EXP table off the
    critical path.
  - samples interleaved: s1 phase-1 chunks pad s0's gd tail and s0's
    out-phase evacuation drain.
"""

import sys

if "/opt/trn_rl_repo" not in sys.path:
    sys.path.insert(0, "/opt/trn_rl_repo")

import numpy as np

import concourse.bacc as bacc
import concourse.tile as tile
from concourse import mybir
from concourse.bass_utils import run_bass_kernel_spmd

N_CORES = 8
B_GLOBAL = 16
B_LOC = B_GLOBAL // N_CORES
C_IN, C_M, C_N = 512, 128, 128
H = W = 64
N = H * W                      # 4096 spatial positions
KC = C_IN // 128               # 4 contraction chunks
NG = 8                         # 8 n-chunks of 512 positions
SHIFT = float(-12.0 * np.log(2.0))   # exp downshift so fp16 never overflows
F32 = mybir.dt.float32
F16 = mybir.dt.float16
EXP = mybir.ActivationFunctionType.Exp
IDENT = mybir.ActivationFunctionType.Identity
AXX = mybir.AxisListType.X


def _build(has_bias_abv: bool, has_bias_r: bool):
    nc = bacc.Bacc("TRN2", target_bir_lowering=False, debug=False)

    # x / out packed host-side as [s, g, p, k, n']: 4KB-contiguous rows.
    x_d = nc.dram_tensor("x", (B_LOC, NG, 128, KC, 512), F16, kind="ExternalInput")
    # weights packed so partition rows are contiguous too
    wab_d = nc.dram_tensor("wab", (128, KC, 256), F16, kind="ExternalInput")
    wvt_d = nc.dram_tensor("wvt", (128, KC, 128), F16, kind="ExternalInput")
    wrt_d = nc.dram_tensor("wrt", (128, C_IN), F16, kind="ExternalInput")
    if has_bias_abv:
        bab_d = nc.dram_tensor("bab", (1, 256), F16, kind="ExternalInput")
        bvt_d = nc.dram_tensor("bvt", (128, 1), F32, kind="ExternalInput")
    if has_bias_r:
        brt_d = nc.dram_tensor("brt", (128, KC), F32, kind="ExternalInput")
    out_d = nc.dram_tensor("out", (B_LOC, NG, 128, KC, 512), F16, kind="ExternalOutput")

    with tile.TileContext(nc) as tc:
        with (
            tc.tile_pool(name="const", bufs=1) as constp,
            tc.tile_pool(name="xq", bufs=B_LOC * NG) as xqp,
            tc.tile_pool(name="mwb", bufs=5) as mwp,
            tc.tile_pool(name="at", bufs=5) as atp,
            tc.tile_pool(name="st", bufs=6) as stp,
            tc.tile_pool(name="ev", bufs=2) as evp,
            tc.tile_pool(name="wn", bufs=2) as wnp,
            tc.tile_pool(name="rw", bufs=4) as rwp,
            tc.tile_pool(name="gds", bufs=2) as gdsp,
            tc.tile_pool(name="gts", bufs=2) as gtsp,
            tc.tile_pool(name="osb", bufs=4) as osbp,
            tc.tile_pool(name="pP", bufs=2, space="PSUM") as pP,
            tc.tile_pool(name="pV", bufs=1, space="PSUM") as pV,
            tc.tile_pool(name="pG", bufs=1, space="PSUM") as pG,
            tc.tile_pool(name="pO", bufs=2, space="PSUM") as pO,
        ):
            # ---- warmup consts (no DMA deps) ----
            ones128 = constp.tile([128, 128], F16)
            nc.gpsimd.memset(ones128[:], 1.0)
            shiftv = constp.tile([128, 1], F32)
            nc.gpsimd.memset(shiftv[:], SHIFT)
            warm16 = constp.tile([128, 512], F16)
            nc.gpsimd.memset(warm16[:], 0.0)

            # ---- loads: first chunk on scalar queue, rest on sync ----
            xqs = [[None] * NG for _ in range(B_LOC)]

            def load_chunk(s, g, eng):
                t = xqp.tile([128, KC, 512], F16, tag="xq", name=f"xq{s}_{g}")
                eng.dma_start(t[:], x_d[s, g])
                xqs[s][g] = t

            # weights stream on sync's ring; the first two x chunks go on
            # scalar's ring in parallel (both rings start ~8.8-10.2us
            # after the fixed framework prologue).  Measured repeatedly:
            # putting x chunk 0 ahead of the weights on sync is ~3us SLOWER.
            load_chunk(0, 0, nc.scalar)
            wab = constp.tile([128, KC, 256], F16)
            nc.sync.dma_start(wab[:], wab_d[:])
            wvt = constp.tile([128, KC, 128], F16)
            nc.sync.dma_start(wvt[:], wvt_d[:])
            load_chunk(0, 1, nc.scalar)
            wrt = constp.tile([128, C_IN], F16)
            nc.sync.dma_start(wrt[:], wrt_d[:])
            for g in range(2, NG):
                load_chunk(0, g, nc.sync)
            for g in range(NG):
                load_chunk(1, g, nc.sync)

            if has_bias_abv:
                bab = constp.tile([1, 256], F16)
                nc.sync.dma_start(bab[:], bab_d[:])
                ones1 = constp.tile([1, 128], F16)
                nc.gpsimd.memset(ones1[:], 1.0)
                bvt = constp.tile([128, 1], F32)
                nc.sync.dma_start(bvt[:], bvt_d[:])
                shiftbv = constp.tile([128, 1], F32)
                nc.vector.tensor_scalar_add(shiftbv[:], bvt[:], SHIFT)
                vbias = shiftbv
            else:
                vbias = shiftv
            if has_bias_r:
                brt = constp.tile([128, KC], F32)
                nc.sync.dma_start(brt[:], brt_d[:])

            # ---- PE p-state warmup + EXP table preload during the DMA
            # wait (PE is free to run ~7.5us in; x chunk 0 lands ~11us)
            warmp = pG.tile([128, 512], F32, tag="pg", name="warmp")

            def dummy_mms(n, target=None):
                t = warmp if target is None else target
                for _ in range(n):
                    nc.tensor.matmul(t[:], ones128[:], warm16[:],
                                     start=True, stop=True)

            dummy_mms(14)
            wexp = constp.tile([128, 1], F16)
            nc.scalar.activation(wexp[:], shiftv[:], EXP, bias=0.0)

            # per-sample state
            mwB = [[None] * NG, [None] * NG]   # exp(B) fp16 [128, 4, 128]
            ats = [[None] * NG, [None] * NG]   # A/sumB fp16 [128, 4, 128]
            evt = [None, None]                 # exp(V^T) fp16 [128, NG, 512]
            wnt = [None, None]                 # normalized W^T fp16
            gdt = [None, None]                 # gd psum
            gts = [None, None]                 # G^T fp16 [128, 512]

            def emit_quad(s, g):
                """A|B projections for the 4 tiles of chunk g + softmax-B."""
                xq = xqs[s][g]
                p1 = pP.tile([128, 1024], F32, tag="pp", name=f"pp{s}_{g}")
                p1v = p1.rearrange("p (t a c) -> p t a c", t=4, c=128)
                for t in range(4):
                    dst = p1[:, t * 256:t * 256 + 256]
                    for k in range(KC):
                        nc.tensor.matmul(
                            dst, xq[:, k, t * 128:t * 128 + 128], wab[:, k, :],
                            start=(k == 0),
                            stop=(k == KC - 1 and not has_bias_abv),
                        )
                    if has_bias_abv:
                        nc.tensor.matmul(dst, ones1[:], bab[:],
                                         start=False, stop=True)
                # one wide exp + one wide reduce + one wide A-scale
                mw = mwp.tile([128, 4, 128], F16, tag="mwb", name=f"mw{s}_{g}")
                nc.scalar.activation(mw[:], p1v[:, :, 1, :], EXP, bias=shiftv[:])
                sums = stp.tile([128, 4, 1], F32, tag="sums")
                nc.vector.tensor_reduce(sums[:], mw[:], axis=AXX,
                                        op=mybir.AluOpType.add)
                rec = stp.tile([128, 4, 1], F32, tag="rec")
                nc.vector.reciprocal(rec[:], sums[:])
                at = atp.tile([128, 4, 128], F16, tag="at", name=f"at{s}_{g}")
                nc.vector.tensor_mul(at[:], p1v[:, :, 0, :],
                                     rec[:].broadcast_to([128, 4, 128]))
                mwB[s][g] = mw
                ats[s][g] = at

            def emit_vt(s, g):
                """V^T [ch, pos] projection for n-chunk g + exp."""
                pv = pV.tile([128, 512], F32, tag="pv", name=f"pv{s}_{g}")
                for k in range(KC):
                    nc.tensor.matmul(pv[:], wvt[:, k, :], xqs[s][g][:, k, :],
                                     start=(k == 0), stop=(k == KC - 1))
                nc.scalar.activation(evt[s][:, g, :], pv[:], EXP, bias=vbias[:])

            def emit_vsum(s, g):
                """Partition sums of exp(V^T) via ones-matmul; recip; Wn."""
                ps = pO.tile([128, 512], F32, tag="po", name=f"ps{s}_{g}")
                nc.tensor.matmul(ps[:], ones128[:], evt[s][:, g, :],
                                 start=True, stop=True)
                rw = rwp.tile([128, 512], F32, tag="rw", name=f"rw{s}_{g}")
                nc.vector.reciprocal_approx_fast(rw[:], ps[:])
                # fp16 SBUF-only multiply on the otherwise-idle gpsimd
                nc.gpsimd.tensor_mul(wnt[s][:, g, :], evt[s][:, g, :], rw[:])

            def emit_gd(s, g, first, last):
                for t in range(4):
                    nc.tensor.matmul(
                        gdt[s][:], ats[s][g][:, t, :], mwB[s][g][:, t, :],
                        start=(first and t == 0), stop=(last and t == 3),
                        skip_group_check=True,
                    )

            def emit_chunk(s, g):
                """All phase-1 work tied to x chunk g of sample s."""
                if g == 0:
                    evt[s] = evp.tile([128, NG, 512], F16, tag="ev",
                                      name=f"ev{s}")
                    wnt[s] = wnp.tile([128, NG, 512], F16, tag="wn",
                                      name=f"wn{s}")
                emit_quad(s, g)
                emit_vt(s, g)
                if g >= 1:
                    emit_vsum(s, g - 1)
                if g >= 2:
                    if g == 2:
                        gdt[s] = pG.tile([128, 128], F32, tag="pg",
                                         name=f"gd{s}")
                    emit_gd(s, g - 2, first=(g == 2), last=False)

            warm2 = [None]

            def emit_tail(s):
                """Finish gd, compute G^T = gd^T @ wR^T."""
                emit_vsum(s, NG - 1)
                emit_gd(s, NG - 2, first=False, last=False)
                emit_gd(s, NG - 1, first=False, last=True)
                gdts = gdsp.tile([128, 128], F16, tag="gds", name=f"gds{s}")
                nc.vector.tensor_copy(gdts[:], gdt[s][:])
                gtp = pO.tile([128, 512], F32, tag="po", name=f"gtp{s}")
                nc.tensor.matmul(gtp[:], gdts[:], wrt[:], start=True, stop=True)
                gt = gtsp.tile([128, 512], F16, tag="gts", name=f"gt{s}")
                nc.scalar.copy(gt[:], gtp[:])
                gts[s] = gt

            def bias_evac(dst, ock, b):
                nc.scalar.activation(dst, ock, IDENT, bias=brt[:, b:b + 1])

            def emit_out(s, gs):
                """out chunks (interleaved with s1 phase-1 while s=0):
                4 c-block matmuls, evac split b0/b1->scalar b2/b3->DVE,
                rotating over 4 narrow psum slots (pG owned by s1's gd)."""
                for g in gs:
                    osb = osbp.tile([128, KC, 512], F16, tag="osb",
                                    name=f"osb{s}_{g}")
                    for b in range(KC):
                        pool, ptag = ((pO, "po"), (pV, "pv"),
                                      (pO, "po"), (pV, "pv"))[b]
                        ock = pool.tile([128, 512], F32, tag=ptag,
                                        name=f"ock{s}_{g}_{b}")
                        nc.tensor.matmul(ock[:], gts[s][:, b * 128:b * 128 + 128],
                                         wnt[s][:, g, :], start=True, stop=True)
                        dst = osb[:, b, :]
                        if has_bias_r:
                            bias_evac(dst, ock[:], b)
                        elif b < 3:
                            # 3:1 scalar-heavy split: in the interleaved
                            # region DVE already carries the phase-1 chain
                            # (reduce+recip+at+recw) and would pace it
                            nc.scalar.copy(dst, ock[:])
                        else:
                            nc.vector.tensor_copy(dst, ock[:])
                    nc.sync.dma_start(out_d[s, g], osb[:])

            def emit_out_final(s):
                """Final out phase (nothing left to interleave): narrow
                evacs split b0/b1->scalar b2/b3->DVE; gd is done so pG
                serves as the 4th rotating psum slot.  The last chunk
                stores b-granular to cut the drain tail."""
                for g in range(NG):
                    osb = osbp.tile([128, KC, 512], F16, tag="osb",
                                    name=f"osb{s}_{g}")
                    for b in range(KC):
                        pool, ptag = ((pO, "po"), (pV, "pv"),
                                      (pO, "po"), (pG, "pg"))[b]
                        ock = pool.tile([128, 512], F32, tag=ptag,
                                        name=f"ock{s}_{g}_{b}")
                        nc.tensor.matmul(ock[:], gts[s][:, b * 128:b * 128 + 128],
                                         wnt[s][:, g, :], start=True, stop=True)
                        dst = osb[:, b, :]
                        if has_bias_r:
                            bias_evac(dst, ock[:], b)
                        elif b < 2:
                            nc.scalar.copy(dst, ock[:])
                        else:
                            nc.vector.tensor_copy(dst, ock[:])
                    if g < NG - 1:
                        nc.sync.dma_start(out_d[s, g], osb[:])
                    else:
                        nc.sync.dma_start(out_d[s, g, :, 0:2], osb[:, 0:2, :])
                        nc.sync.dma_start(out_d[s, g, :, 2:3], osb[:, 2:3, :])
                        nc.sync.dma_start(out_d[s, g, :, 3:4], osb[:, 3:4, :])

            # ---- schedule: s1 phase-1 interleaves s0's out phase ----
            for g in range(NG):
                emit_chunk(0, g)
            emit_chunk(1, 0)          # pads s0's gd-tail wait
            emit_tail(0)
            emit_chunk(1, 1)          # pads s0's G -> gts chain
            for g in range(2, NG):
                if g >= 3:
                    emit_out(0, [g - 3])
                emit_chunk(1, g)
            emit_out(0, [NG - 3])     # these two pad s1's at(7) softmax
            emit_out(0, [NG - 2])     # chain (~2.3us) before the gd tail
            emit_tail(1)
            emit_out(0, [NG - 1])     # pads s1's G -> gts chain
            emit_out_final(1)

    nc.compile()
    return nc


_CACHE = {}


def _get_nc(has_bias_abv: bool, has_bias_r: bool):
    key = (has_bias_abv, has_bias_r)
    if key not in _CACHE:
        _CACHE[key] = _build(*key)
    return _CACHE[key]


def _run(inputs, trace=False, **spmd_kwargs):
    x = np.asarray(inputs["x"])
    b, c, h, w = x.shape
    assert (b, c, h, w) == (B_GLOBAL, C_IN, H, W), x.shape
    wA = np.asarray(inputs["wA"], np.float32)
    wB = np.asarray(inputs["wB"], np.float32)
    wV = np.asarray(inputs["wV"], np.float32)
    wR = np.asarray(inputs["wR"], np.float32)
    bA = np.asarray(inputs["bA"], np.float32)
    bB = np.asarray(inputs["bB"], np.float32)
    bV = np.asarray(inputs["bV"], np.float32)
    bR = np.asarray(inputs["bR"], np.float32)

    has_bias_abv = bool(np.any(bA) or np.any(bB) or np.any(bV))
    has_bias_r = bool(np.any(bR))
    nc = _get_nc(has_bias_abv, has_bias_r)

    # [128, KC, 256]: row p, chunk k holds [wA.T | wB.T][k*128+p, :]
    wab = np.concatenate([wA.T, wB.T], axis=1).reshape(KC, 128, 256)
    wvt = wV.T.reshape(KC, 128, 128)
    base = {
        "wab": np.ascontiguousarray(wab.transpose(1, 0, 2), dtype=np.float16),
        "wvt": np.ascontiguousarray(wvt.transpose(1, 0, 2), dtype=np.float16),
        "wrt": np.ascontiguousarray(wR.T, dtype=np.float16),
    }
    if has_bias_abv:
        base["bab"] = np.concatenate([bA, bB])[None, :].astype(np.float16)
        base["bvt"] = np.ascontiguousarray(bV[:, None], np.float32)
    if has_bias_r:
        base["brt"] = np.ascontiguousarray(bR.reshape(KC, 128).T, np.float32)

    # pack x: [B, 512, 4096] -> [B, g, p, k, n']  (4KB contiguous rows)
    xh = (
        np.asarray(x, np.float16)
        .reshape(B_GLOBAL, KC, 128, NG, 512)
        .transpose(0, 3, 2, 1, 4)
    )
    in_maps = [
        dict(base, x=np.ascontiguousarray(
            xh[ci * B_LOC:(ci + 1) * B_LOC]))
        for ci in range(N_CORES)
    ]
    res = run_bass_kernel_spmd(
        nc, in_maps, core_ids=list(range(N_CORES)), trace=trace, **spmd_kwargs
    )
    # unpack out: [B_LOC, g, p, k, n'] -> [B_LOC, 512, 4096]
    outs = []
    for ci in range(N_CORES):
        o = res.results[ci]["out"].astype(np.float32)
        outs.append(o.transpose(0, 3, 2, 1, 4).reshape(B_LOC, C_IN, N))
    out = np.concatenate(outs, axis=0)
    return out.reshape(B_GLOBAL, C_IN, H, W), res


def kernel(**inputs):
    out, _ = _run(inputs)
    return out
